# revision 1
# baseline (speedup 1.0000x reference)
"""Nystrom attention Trainium2 kernel.

Sharding: 8 cores = 4 batches x 2 head-groups (4 heads each).
Each core computes its (batch, head-group) slice end-to-end, including its
share of the output projection; the host sums the two partial output
projections per batch and adds bo.

Device layouts (per core, one SPMD program):
  xT   [512, 8192]  bf16   x[b] transposed (emb on partitions)
  wqk  [512, 512]   bf16   [Wq_heads*scale | Wk_heads*scale]
  wv   [512, 256]   bf16
  wo   [256, 512]   bf16   Wo rows for this head group
  bqk  [512]        f32    [bq_heads | bk_heads]*scale
  bvb  [128, 256]   f32    bv broadcast to 128 partitions
  idbf [128, 128]   bf16   identity (PE transpose)
  idf32[128, 128]   f32    identity
  nsc  [64, 192]    f32    [7*I | 15*I | 3.25*I]
  onesr[1, 128]     f32    ones row
Output:
  out  [8192, 512]  f32    partial (x@..@Wo for this head group), no bo
"""

import numpy as np
import ml_dtypes

import concourse.bass as bass
import concourse.tile as tile
from concourse import bacc, mybir
from concourse.bass_utils import run_bass_kernel_spmd

BF16 = mybir.dt.bfloat16
F32 = mybir.dt.float32
AF = mybir.ActivationFunctionType
AX = mybir.AxisListType
OP = mybir.AluOpType

S = 8192        # sequence length
E = 512         # embedding dim
D = 64          # head dim
L = 64          # landmarks
NHG = 4         # heads per core (head group)
N_ITER = 6
SCALE = 1.0 / np.sqrt(np.sqrt(D))

_CACHED_NC = None


def _build(phases=(1, 2, 3)):
    nc = bacc.Bacc("TRN2", target_bir_lowering=False, debug=False, num_devices=8)

    xT_d = nc.dram_tensor("xT", [E, S], BF16, kind="ExternalInput").ap()
    wqk_d = nc.dram_tensor("wqk", [E, 512], BF16, kind="ExternalInput").ap()
    wv_d = nc.dram_tensor("wv", [E, 256], BF16, kind="ExternalInput").ap()
    wo_d = nc.dram_tensor("wo", [256, E], BF16, kind="ExternalInput").ap()
    bqk_d = nc.dram_tensor("bqk", [512], F32, kind="ExternalInput").ap()
    bvb_d = nc.dram_tensor("bvb", [128, 256], F32, kind="ExternalInput").ap()
    idbf_d = nc.dram_tensor("idbf", [128, 128], BF16, kind="ExternalInput").ap()
    idf32_d = nc.dram_tensor("idf32", [128, 128], F32, kind="ExternalInput").ap()
    nsc_d = nc.dram_tensor("nsc", [64, 192], F32, kind="ExternalInput").ap()
    onesr_d = nc.dram_tensor("onesr", [1, 128], F32, kind="ExternalInput").ap()
    blk1_d = nc.dram_tensor("blk1", [128, 128], BF16,
                            kind="ExternalInput").ap()
    out_d = nc.dram_tensor("out", [S, E], F32, kind="ExternalOutput").ap()

    with tile.TileContext(nc) as tc:
        _emit(nc, tc, xT_d, wqk_d, wv_d, wo_d, bqk_d, bvb_d, idbf_d, idf32_d,
              nsc_d, onesr_d, blk1_d, out_d, phases)
    nc.compile()
    return nc


def _emit(nc, tc, xT_d, wqk_d, wv_d, wo_d, bqk_d, bvb_d, idbf_d, idf32_d,
          nsc_d, onesr_d, blk1_d, out_d, phases=(1, 2, 3)):
    with (
        tc.tile_pool(name="const", bufs=1) as const,
        tc.tile_pool(name="big", bufs=1) as big,
        tc.tile_pool(name="small", bufs=2) as small,
    ):
        # ---- constants / weights into SBUF ----
        wqk_sb = const.tile([128, 4, 512], BF16, tag="wqk")
        nc.sync.dma_start(wqk_sb[:], wqk_d.rearrange("(ko p) m -> p ko m", p=128))
        wv_sb = const.tile([128, 4, 256], BF16, tag="wv")
        nc.sync.dma_start(wv_sb[:], wv_d.rearrange("(ko p) m -> p ko m", p=128))
        wo_sb = const.tile([128, 2, 512], BF16, tag="wo")
        nc.sync.dma_start(wo_sb[:], wo_d.rearrange("(j p) m -> p j m", p=128))
        bqk_sb = const.tile([128, 4], F32, tag="bqk")
        nc.sync.dma_start(bqk_sb[:], bqk_d.rearrange("(t p) -> p t", p=128))
        bvb_sb = const.tile([128, 256], F32, tag="bvb")
        nc.sync.dma_start(bvb_sb[:], bvb_d[:])
        idbf_sb = const.tile([128, 128], BF16, tag="idbf")
        nc.sync.dma_start(idbf_sb[:], idbf_d[:])
        idf32_sb = const.tile([128, 128], F32, tag="idf32")
        nc.sync.dma_start(idf32_sb[:], idf32_d[:])
        nsc_sb = const.tile([64, 192], F32, tag="nsc")
        nc.sync.dma_start(nsc_sb[:], nsc_d[:])
        onesr_sb = const.tile([1, 128], F32, tag="onesr")
        nc.sync.dma_start(onesr_sb[:], onesr_d[:])
        blk1_sb = const.tile([128, 128], BF16, tag="blk1")
        nc.sync.dma_start(blk1_sb[:], blk1_d[:])

        # ---- persistent activations ----
        qT = big.tile([128, 2, S], BF16, tag="qT")      # (2h d | seq), per hp
        kT = big.tile([128, 2, S], BF16, tag="kT")
        vsb = big.tile([128, 64, 4, 65], BF16, tag="v")  # (s | chunk, head, d+1)
        landq = const.tile([128, 2, L], F32, tag="landq")  # raw segment sums
        landk = const.tile([128, 2, L], F32, tag="landk")

        nc.vector.memset(vsb[:, :, :, 64:65], 1.0)

        xT_t = xT_d.rearrange("(ko p) s -> p ko s", p=128)

        # ================= Phase 1: QKV projection =================
        if 1 not in phases:
            pass
        else:
         with (
            tc.tile_pool(name="xt", bufs=4) as xpool,
            tc.tile_pool(name="ps_qk", bufs=5, space="PSUM") as ps_qk,
            tc.tile_pool(name="ps_v", bufs=3, space="PSUM") as ps_v,
        ):
            for c in range(16):
                sl = bass.ts(c, 512)
                xt = xpool.tile([128, 4, 512], BF16, tag="xt")
                nc.sync.dma_start(xt[:], xT_t[:, :, sl])
                for t in range(4):  # q01 q23 k01 k23
                    ps = ps_qk.tile([128, 512], F32, tag="psqk")
                    for ko in range(4):
                        nc.tensor.matmul(
                            ps[:], lhsT=wqk_sb[:, ko, bass.ts(t, 128)],
                            rhs=xt[:, ko, :], start=(ko == 0), stop=(ko == 3))
                    dst = qT if t < 2 else kT
                    hp = t % 2
                    nc.scalar.activation(dst[:, hp, sl], ps[:], AF.Identity,
                                         bias=bqk_sb[:, t:t + 1])
                    land = landq if t < 2 else landk
                    nc.vector.reduce_sum(
                        land[:, hp, bass.ts(c, 4)],
                        dst[:, hp, sl].rearrange("p (g s) -> p g s", s=128),
                        axis=AX.X)
                for s4 in range(4):
                    psv = ps_v.tile([128, 256], F32, tag="psv")
                    for ko in range(4):
                        nc.tensor.matmul(
                            psv[:], lhsT=xt[:, ko, bass.ts(s4, 128)],
                            rhs=wv_sb[:, ko, :], start=(ko == 0), stop=(ko == 3))
                    nc.vector.tensor_tensor(
                        vsb[:, c * 4 + s4, :, 0:64],
                        psv[:].rearrange("p (h d) -> p h d", d=64),
                        bvb_sb[:].rearrange("p (h d) -> p h d", d=64),
                        op=OP.add)

        # ---- landmark means (+bias), cast bf16, build block-diagonals ----
        landq_bf = const.tile([128, 2, L], BF16, tag="landqbf")
        landk_bf = const.tile([128, 2, L], BF16, tag="landkbf")
        # qT/kT already carry the bias, so only the 1/seg scale here
        for hp in range(2):
            nc.vector.tensor_scalar_mul(landq_bf[:, hp, :], landq[:, hp, :],
                                        1.0 / 128.0)
            nc.vector.tensor_scalar_mul(landk_bf[:, hp, :], landk[:, hp, :],
                                        1.0 / 128.0)
        qblk = []
        kblk = []
        for hp in range(2):
            qb = const.tile([128, 128], BF16, tag=f"qblk{hp}")
            kb = const.tile([128, 128], BF16, tag=f"kblk{hp}")
            for b_ in (qb, kb):
                nc.vector.memset(b_[:], 0.0)
            nc.vector.tensor_copy(qb[0:64, 0:64], landq_bf[0:64, hp, :])
            nc.vector.tensor_copy(qb[64:128, 64:128], landq_bf[64:128, hp, :])
            nc.vector.tensor_copy(kb[0:64, 0:64], landk_bf[0:64, hp, :])
            nc.vector.tensor_copy(kb[64:128, 64:128], landk_bf[64:128, hp, :])
            qblk.append(qb)
            kblk.append(kb)

        # block-diagonal t2 (lhsT of the hcT matmul), built later
        t2blk = []
        for hp in range(2):
            tb = const.tile([128, 128], BF16, tag=f"t2blk{hp}")
            nc.vector.memset(tb[:], 0.0)
            t2blk.append(tb)

        # ============ Phase 2a: kernel_3 -> t1 accumulation ============
        K2 = const.tile([128, L], F32, tag="K2")  # both heads of hp stacked
        K2s = [K2]
        K2b = const.tile([128, L], F32, tag="K2b")
        K2s.append(K2b)
        if 2 in phases:
         with tc.tile_pool(name="ps_t1", bufs=4, space="PSUM") as ps_t1:
            t1ps = [ps_t1.tile([65, 64], F32, tag="t1", name=f"t1ps{i}")
                    for i in range(4)]
            with (
                tc.tile_pool(name="ps_s3", bufs=4, space="PSUM") as ps_s3,
                tc.tile_pool(name="e3p", bufs=3) as e3p,
            ):
                for cg in range(16):
                    for hp in range(2):
                        ps3 = ps_s3.tile([128, 512], F32, tag="ps3")
                        for i in range(4):
                            nc.tensor.matmul(
                                ps3[:, bass.ts(i, 128)],
                                lhsT=kT[:, hp, bass.ts(cg * 4 + i, 128)],
                                rhs=qblk[hp], start=True, stop=True)
                        e3 = e3p.tile([128, 512], BF16, tag="e3")
                        nc.scalar.activation(e3[:], ps3[:], AF.Exp)
                        for i in range(4):
                            c = cg * 4 + i
                            for h2 in range(2):
                                h = hp * 2 + h2
                                nc.tensor.matmul(
                                    t1ps[h][:], lhsT=vsb[:, c, h, :],
                                    rhs=e3[:, i * 128 + h2 * 64:
                                           i * 128 + h2 * 64 + 64],
                                    start=(c == 0), stop=(c == 63),
                                    skip_group_check=True)
                # kernel_2 (landmark x landmark) while ps_s3 still open
                for hp in range(2):
                    ps2 = ps_s3.tile([128, 128], F32, tag="ps3")
                    nc.tensor.matmul(ps2[:], lhsT=qblk[hp], rhs=kblk[hp],
                                     start=True, stop=True)
                    k2e = small.tile([128, L], F32, tag="k2e")
                    rs = small.tile([128, 1], F32, tag="k2rs")
                    for h2 in range(2):
                        nc.scalar.activation(k2e[bass.ts(h2, 64), :],
                                             ps2[bass.ts(h2, 64), bass.ts(h2, 64)],
                                             AF.Exp,
                                             accum_out=rs[bass.ts(h2, 64), :])
                    ri = small.tile([128, 1], F32, tag="k2ri")
                    nc.vector.reciprocal(ri[:], rs[:])
                    nc.vector.tensor_scalar_mul(K2s[hp][:], k2e[:], ri[:])

            # ========= Phase 2b: Newton-Schulz inverse + t2 =========
            with (
                tc.tile_pool(name="ps_ns", bufs=4, space="PSUM") as ps_ns,
                tc.tile_pool(name="nsp", bufs=2) as nsp,
            ):
                id64 = idf32_sb[0:64, 0:64]
                HS = [(h, h // 2, h % 2, bass.ts(h % 2, 64)) for h in range(4)]
                K2T = {}
                mxi = {}
                V = {}
                W = {}
                for h, hp, h2, psl in HS:
                    K2h = K2s[hp][psl, :]
                    pk = ps_ns.tile([65, 65], F32, tag="ns", name=f"pk{h}")
                    nc.tensor.transpose(pk[0:64, 0:64], K2h,
                                        idf32_sb[psl, psl])
                    K2T[h] = nsp.tile([64, 64], F32, tag=f"K2T{h}",
                                      name=f"K2T{h}")
                    nc.vector.tensor_copy(K2T[h][:], pk[0:64, 0:64])
                for h, hp, h2, psl in HS:
                    # max column-sum of K2  (= max row-sum of K2T)
                    cs = nsp.tile([64, 1], F32, tag=f"cs{h}", name=f"cs{h}")
                    nc.vector.reduce_sum(cs[:], K2T[h][:], axis=AX.X)
                    pc = ps_ns.tile([65, 65], F32, tag="ns", name=f"pc{h}")
                    nc.tensor.transpose(pc[0:1, 0:64], cs[:], id64)
                    mx = nsp.tile([1, 1], F32, tag=f"mx{h}", name=f"mx{h}")
                    nc.vector.reduce_max(mx[:], pc[0:1, 0:64], axis=AX.X)
                    pb = ps_ns.tile([65, 65], F32, tag="ns", name=f"pb{h}")
                    nc.tensor.matmul(pb[0:64, 0:1], lhsT=onesr_sb[0:1, 0:64],
                                     rhs=mx[:], start=True, stop=True)
                    mxi[h] = nsp.tile([64, 1], F32, tag=f"mxi{h}",
                                      name=f"mxi{h}")
                    nc.vector.reciprocal(mxi[h][:], pb[0:64, 0:1])
                for h, hp, h2, psl in HS:
                    V[h] = nsp.tile([64, 64], F32, tag=f"V{h}", name=f"V{h}")
                    nc.vector.tensor_scalar_mul(V[h][:], K2T[h][:], mxi[h][:])
                    W[h] = nsp.tile([64, 64], F32, tag=f"W{h}", name=f"W{h}")
                    nc.vector.tensor_scalar_mul(W[h][:], K2s[hp][psl, :],
                                                mxi[h][:])

                for _ in range(N_ITER):
                    pkv = {}
                    T1 = {}
                    KVT = {}
                    T2 = {}
                    T3 = {}
                    for h, hp, h2, psl in HS:
                        p = ps_ns.tile([65, 65], F32, tag="ns", name=f"pkv{h}")
                        nc.tensor.matmul(p[0:64, 0:64], lhsT=K2T[h][:],
                                         rhs=V[h][:], start=True, stop=True)
                        pkv[h] = p
                    for h, hp, h2, psl in HS:
                        T1[h] = nsp.tile([64, 64], F32, tag=f"T1{h}",
                                         name=f"T1{h}")
                        nc.vector.tensor_tensor(T1[h][:], nsc_sb[:, 0:64],
                                                pkv[h][0:64, 0:64],
                                                op=OP.subtract)
                        p = ps_ns.tile([65, 65], F32, tag="ns", name=f"pvt{h}")
                        nc.tensor.matmul(p[0:64, 0:64], lhsT=V[h][:],
                                         rhs=K2T[h][:], start=True, stop=True)
                        KVT[h] = nsp.tile([64, 64], F32, tag=f"KVT{h}",
                                          name=f"KVT{h}")
                        nc.vector.tensor_copy(KVT[h][:], p[0:64, 0:64])
                    for h, hp, h2, psl in HS:
                        p = ps_ns.tile([65, 65], F32, tag="ns", name=f"p3{h}")
                        nc.tensor.matmul(p[0:64, 0:64], lhsT=KVT[h][:],
                                         rhs=T1[h][:], start=True, stop=True)
                        T2[h] = nsp.tile([64, 64], F32, tag=f"T2{h}",
                                         name=f"T2{h}")
                        nc.vector.tensor_tensor(T2[h][:], nsc_sb[:, 64:128],
                                                p[0:64, 0:64], op=OP.subtract)
                    for h, hp, h2, psl in HS:
                        p = ps_ns.tile([65, 65], F32, tag="ns", name=f"p4{h}")
                        nc.tensor.matmul(p[0:64, 0:64], lhsT=KVT[h][:],
                                         rhs=T2[h][:], start=True, stop=True)
                        T3[h] = nsp.tile([64, 64], F32, tag=f"T3{h}",
                                         name=f"T3{h}")
                        nc.vector.scalar_tensor_tensor(
                            T3[h][:], p[0:64, 0:64], -0.25,
                            nsc_sb[:, 128:192], op0=OP.mult, op1=OP.add)
                    for h, hp, h2, psl in HS:
                        p5 = ps_ns.tile([65, 65], F32, tag="ns", name=f"p5{h}")
                        nc.tensor.matmul(p5[0:64, 0:64], lhsT=W[h][:],
                                         rhs=T3[h][:], start=True, stop=True)
                        p6 = ps_ns.tile([65, 65], F32, tag="ns", name=f"p6{h}")
                        nc.tensor.matmul(p6[0:64, 0:64], lhsT=T3[h][:],
                                         rhs=W[h][:], start=True, stop=True)
                        V[h] = nsp.tile([64, 64], F32, tag=f"V{h}",
                                        name=f"V{h}")
                        nc.vector.tensor_copy(V[h][:], p5[0:64, 0:64])
                        W[h] = nsp.tile([64, 64], F32, tag=f"W{h}",
                                        name=f"W{h}")
                        nc.vector.tensor_copy(W[h][:], p6[0:64, 0:64])

                t1n = {}
                for h, hp, h2, psl in HS:
                    # t1 normalize: transpose [65,64] -> [64,65]
                    t1u = nsp.tile([65, 64], F32, tag=f"t1u{h}", name=f"t1u{h}")
                    nc.vector.tensor_copy(t1u[:], t1ps[h][:])
                    ptt = ps_ns.tile([65, 65], F32, tag="ns", name=f"ptt{h}")
                    nc.tensor.transpose(ptt[0:64, 0:65], t1u[:],
                                        idf32_sb[0:65, 0:65])
                    d3i = nsp.tile([64, 1], F32, tag=f"d3i{h}", name=f"d3i{h}")
                    nc.vector.reciprocal(d3i[:], ptt[0:64, 64:65])
                    t1n[h] = nsp.tile([64, 64], F32, tag=f"t1n{h}",
                                      name=f"t1n{h}")
                    nc.vector.tensor_scalar_mul(t1n[h][:], ptt[0:64, 0:64],
                                                d3i[:])
                for h, hp, h2, psl in HS:
                    # t2 = V2 @ t1n  (lhsT = W = V2^T)
                    pt2 = ps_ns.tile([65, 65], F32, tag="ns", name=f"pt2{h}")
                    nc.tensor.matmul(pt2[0:64, 0:64], lhsT=W[h][:],
                                     rhs=t1n[h][:], start=True, stop=True)
                    nc.vector.tensor_copy(t2blk[hp][psl, psl],
                                          pt2[0:64, 0:64])

        # ======= Phase 3: kernel_1, apply, output projection =======
        if 3 not in phases:
            return
        with (
            tc.tile_pool(name="ps_s1", bufs=2, space="PSUM") as ps_s1,
            tc.tile_pool(name="ps_rb", bufs=2, space="PSUM") as ps_rb,
            tc.tile_pool(name="ps_ht", bufs=2, space="PSUM") as ps_ht,
            tc.tile_pool(name="ps_out", bufs=2, space="PSUM") as ps_out,
            tc.tile_pool(name="e1p", bufs=3) as e1p,
            tc.tile_pool(name="hcp", bufs=3) as hcp,
        ):
            for c in range(16):
                hcts = []
                for hp in range(2):
                    ps1 = ps_s1.tile([128, 512], F32, tag="ps1")
                    nc.tensor.matmul(ps1[:], lhsT=kblk[hp],
                                     rhs=qT[:, hp, bass.ts(c, 512)],
                                     start=True, stop=True)
                    e1t = e1p.tile([128, 512], BF16, tag="e1")
                    nc.scalar.activation(e1t[:], ps1[:], AF.Exp)
                    e1 = e1t[:]
                    # per-head kernel_1 row-sums, pre-broadcast to the
                    # (head, d) partition layout via block-ones matmul
                    prb = ps_rb.tile([128, 512], F32, tag="prb")
                    nc.tensor.matmul(prb[:], lhsT=blk1_sb[:], rhs=e1[:],
                                     start=True, stop=True)
                    rbs = e1p.tile([128, 512], F32, tag="rbs")
                    nc.vector.reciprocal(rbs[:], prb[:])
                    # hcT (unnormalized) = blockdiag(t2).T @ e1 -> [(h,d), s]
                    pht = ps_ht.tile([128, 512], F32, tag="pht")
                    nc.tensor.matmul(pht[:], lhsT=t2blk[hp], rhs=e1[:],
                                     start=True, stop=True)
                    hct = hcp.tile([128, 512], BF16, tag="hct")
                    nc.vector.tensor_tensor(hct[:], pht[:], rbs[:],
                                            op=OP.mult)
                    hcts.append(hct)
                for s4 in range(4):
                    c128 = c * 4 + s4
                    pso2 = ps_out.tile([128, 512], F32, tag="psout")
                    for hp in range(2):
                        nc.tensor.matmul(pso2[:],
                                         lhsT=hcts[hp][:, bass.ts(s4, 128)],
                                         rhs=wo_sb[:, hp, :],
                                         start=(hp == 0), stop=(hp == 1))
                    osb = hcp.tile([128, 512], F32, tag="osb")
                    nc.scalar.copy(osb[:], pso2[:])
                    nc.sync.dma_start(out_d[bass.ts(c128, 128), :], osb[:])


def _prep_inputs(x, Wq, bq, Wk, bk, Wv, bv, Wo, bo):
    bf = ml_dtypes.bfloat16
    x = np.asarray(x, dtype=np.float32)
    Wq = np.asarray(Wq, dtype=np.float32)
    Wk = np.asarray(Wk, dtype=np.float32)
    Wv = np.asarray(Wv, dtype=np.float32)
    Wo = np.asarray(Wo, dtype=np.float32)
    bq = np.asarray(bq, dtype=np.float32)
    bk = np.asarray(bk, dtype=np.float32)
    bv = np.asarray(bv, dtype=np.float32)

    idf = np.eye(128, dtype=np.float32)
    consts = {
        "idbf": np.ascontiguousarray(idf.astype(bf)),
        "idf32": idf,
        "nsc": np.ascontiguousarray(np.concatenate(
            [7.0 * np.eye(64), 15.0 * np.eye(64), 3.25 * np.eye(64)],
            axis=1).astype(np.float32)),
        "onesr": np.ones((1, 128), dtype=np.float32),
        "blk1": np.ascontiguousarray(
            np.kron(np.eye(2), np.ones((64, 64))).astype(bf)),
    }
    in_maps = []
    for core in range(8):
        b, g = core // 2, core % 2
        hsl = slice(g * 256, (g + 1) * 256)
        xT = np.ascontiguousarray(x[b].T.astype(bf))
        wqk = np.ascontiguousarray(
            np.concatenate([Wq[:, hsl], Wk[:, hsl]], axis=1) * SCALE).astype(bf)
        wv = np.ascontiguousarray(Wv[:, hsl]).astype(bf)
        wo = np.ascontiguousarray(Wo[hsl, :]).astype(bf)
        bqk = np.ascontiguousarray(
            np.concatenate([bq[hsl], bk[hsl]]) * SCALE).astype(np.float32)
        bvb = np.ascontiguousarray(
            np.broadcast_to(bv[hsl], (128, 256))).astype(np.float32)
        in_maps.append({
            "xT": xT, "wqk": wqk, "wv": wv, "wo": wo,
            "bqk": bqk, "bvb": bvb, **consts,
        })
    return in_maps


def run_on_device(in_maps, **kwargs):
    global _CACHED_NC
    if _CACHED_NC is None:
        _CACHED_NC = _build()
    return run_bass_kernel_spmd(_CACHED_NC, in_maps, core_ids=list(range(8)),
                                **kwargs)


def kernel(x, Wq, bq, Wk, bk, Wv, bv, Wo, bo):
    in_maps = _prep_inputs(x, Wq, bq, Wk, bk, Wv, bv, Wo, bo)
    res = run_on_device(in_maps)
    bo = np.asarray(bo, dtype=np.float32)
    out = np.empty((4, S, E), dtype=np.float32)
    for b in range(4):
        out[b] = res.results[2 * b]["out"] + res.results[2 * b + 1]["out"] + bo
    return out



# revision 41
# speedup vs baseline: 1.9026x; 1.9026x over previous
"""Nystrom attention Trainium2 kernel (fused landmark formulation).

Sharding: 8 cores = 4 batches x 2 head-groups (4 heads each). Each core
computes its (batch, head-group) slice; the host sums the two bf16 partial
output projections per batch (in f32) and adds bo.

Algebra (per head h, SCALE = HEAD_DIM**-0.25, q = x@Wq + bq etc.):
  x_land   = segment means of x (host; linear pooling of the input)
  q_landT  = Wq^T-contract(x_landT*SCALE) + bq*SCALE   [(h,d), L] on device
  logits1  = x @ M1T + bq.k_land         M1T = Wq-contract(k_landT)*SCALE
  logits3  = (x @ M3T)^T-ish             (kernel_3's bias is constant along
                                          its softmax axis; drops out exactly)
  K2       = softmax(q_landT^T k_landT) per head; invK2 via Newton-Schulz.
  v16      = x @ (Wv*16)   (bv folds into t1n; Wo/16 compensates the 16)
  t1       = [v16|1]^T @ exp(logits3) -> t1n = rows/rowsum + bv*16
  m        = ((invK2 @ t1n) @ (Wo/16))^T-chain => m_sb [(h,L), E]
  out      = (e1 / rowsum_head(e1)) @ m,   e1 = exp(logits1)

Big matmuls are fp8e4 DoubleRow (hi/lo compensated for v; logits are tiny
(~0.1 rms) so single fp8 is safe there). Newton-Schulz runs bf16 for
iterations 0-4 and f32 for the final iteration (last-iter precision
dominates the result), with its stages interleaved into phase A.
"""

import os
import numpy as np
import ml_dtypes

import concourse.bass as bass
import concourse.tile as tile
from concourse import bacc, mybir
from concourse.bass_utils import run_bass_kernel_spmd

BF16 = mybir.dt.bfloat16
F32 = mybir.dt.float32
FP8 = mybir.dt.float8e4
AF = mybir.ActivationFunctionType
AX = mybir.AxisListType
OP = mybir.AluOpType
PM = mybir.MatmulPerfMode

S = 8192        # sequence length
E = 512         # embedding dim
D = 64          # head dim
L = 64          # landmarks
NHG = 4         # heads per core (head group)
N_ITER = 6
SCALE = 1.0 / np.sqrt(np.sqrt(D))
M8 = 64.0       # fp8 prescale on M1T/M3T (undone by exp scale)
NSPLIT = 4      # x8/xlo DMA pipelining splits along S
USE_DR = os.environ.get("K_DR", "1") == "1"
USE_GPSIMD = os.environ.get("K_GP", "1") == "1"

_CACHED_NC = None
_TILES = {}


def _build():
    nc = bacc.Bacc("TRN2", target_bir_lowering=False, debug=False, num_devices=8)

    dram = {}
    for name, shape, dt in [
        ("x8", [E, S], FP8),
        ("xlo", [E, S], FP8),
        ("xlT", [E, L], BF16),
        ("wq", [E, 256], BF16),
        ("wk", [E, 256], BF16),
        ("wqT", [256, E], BF16),
        ("wkT", [256, E], BF16),
        ("wv16h", [E, 256], FP8),
        ("wv16l", [E, 256], FP8),
        ("wo16", [256, E], BF16),
        ("bqs", [256], BF16),
        ("bks", [256], BF16),
        ("bv16b", [64, NHG, L], F32),
        ("nsc2", [64, 3, 256], F32),
        ("idf32", [128, 128], F32),
        ("blk1", [128, 128], BF16),
        ("onesr", [1, 128], F32),
    ]:
        dram[name] = nc.dram_tensor(name, shape, dt, kind="ExternalInput").ap()
    out_d = nc.dram_tensor("out", [S, E], BF16, kind="ExternalOutput").ap()

    with tile.TileContext(nc) as tc:
        _emit(nc, tc, dram, out_d)
    nc.compile()
    return nc


def _emit(nc, tc, dram, out_d):
    SP = S // NSPLIT
    with (
        tc.tile_pool(name="const", bufs=1) as const,
        tc.tile_pool(name="big", bufs=1) as big,
        tc.tile_pool(name="small", bufs=2) as small,
    ):
        def load(name, shape, dt, pat=None, **kw):
            t = const.tile(shape, dt, tag=name)
            src = dram[name]
            if pat is not None:
                src = src.rearrange(pat, **kw)
            nc.sync.dma_start(t[:], src)
            return t

        # small consts first so phase 0 can begin immediately
        xlT = load("xlT", [128, 4, L], BF16, "(ko p) l -> p ko l", p=128)
        wq = load("wq", [128, 4, 256], BF16, "(ko p) m -> p ko m", p=128)
        wk = load("wk", [128, 4, 256], BF16, "(ko p) m -> p ko m", p=128)
        wqT = load("wqT", [64, NHG, E], BF16, "(h p) m -> p h m", p=64)
        wkT = load("wkT", [64, NHG, E], BF16, "(h p) m -> p h m", p=64)
        bqs = load("bqs", [64, NHG, 1], BF16, "(h p) -> p h ()", p=64)
        bks = load("bks", [64, NHG, 1], BF16, "(h p) -> p h ()", p=64)
        idf32 = load("idf32", [128, 128], F32)
        onesr = load("onesr", [1, 128], F32)
        wv16h = load("wv16h", [128, 4, 256], FP8, "(ko p) m -> p ko m", p=128)
        wv16l = load("wv16l", [128, 4, 256], FP8, "(ko p) m -> p ko m", p=128)

        # big inputs, split along S; first split lands before the consts that
        # are only needed later (N-S constants, phase A'/B weights) so the
        # streaming phase can start as early as possible.
        def load_split(i):
            ssl = slice(i * SP, (i + 1) * SP)
            t8 = big.tile([128, 4, SP], FP8, tag=f"x8_{i}")
            nc.sync.dma_start(
                t8[:], dram["x8"][:, ssl].rearrange("(ko p) s -> p ko s", p=128))
            tlo = big.tile([128, 4, SP], FP8, tag=f"xlo_{i}")
            nc.sync.dma_start(
                tlo[:], dram["xlo"][:, ssl].rearrange("(ko p) s -> p ko s", p=128))
            return t8, tlo

        x8s, xlos = [], []
        t8, tlo = load_split(0)
        x8s.append(t8)
        xlos.append(tlo)
        nsc2 = load("nsc2", [64, 3, 256], F32)
        bv16b = load("bv16b", [64, NHG, L], F32)
        blk1 = load("blk1", [128, 128], BF16)
        wo16 = load("wo16", [64, NHG, E], BF16, "(h p) m -> p h m", p=64)
        for i in range(1, NSPLIT):
            t8, tlo = load_split(i)
            x8s.append(t8)
            xlos.append(tlo)

        # persistent activations / results
        vsb = big.tile([128, 64, NHG, 65], BF16, tag="v")  # (s | c4, head, d+1)
        nc.vector.memset(vsb[:, :, :, 64:65], 1.0)
        m_sb = const.tile([128, 2, E], BF16, tag="m")      # out-proj matrix
        bias1 = const.tile([128, 2, 1], F32, tag="bias1")  # exp bias per (h,L)

        qlT = const.tile([64, NHG, L], BF16, tag="qlT")  # (d part, head, L)
        klT = const.tile([64, NHG, L], BF16, tag="klT")
        M1T8 = const.tile([128, 4, 256], FP8, tag="M1T8")  # (E part, hL free)
        M3T8 = const.tile([128, 4, 256], FP8, tag="M3T8")

        if os.environ.get("K_NOP0") == "1":
            with tc.tile_pool(name="zz3", bufs=1) as zz:
                z = zz.tile([128, E], BF16, tag="z")
                nc.vector.memset(z[:], 0.0)
                for c128 in range(64):
                    nc.sync.dma_start(out_d[bass.ts(c128, 128), :], z[:])
            return
        # ============ Phase 0: landmark branch + N-S inverse ============
        global _TILES
        _TILES = {"vsb": vsb, "m_sb": m_sb, "qlT": qlT, "klT": klT,
                  "M1T8": M1T8, "M3T8": M3T8, "bias1": bias1}
        W_holder = {}
        with (
            tc.tile_pool(name="ps0", bufs=4, space="PSUM") as ps0,
            tc.tile_pool(name="ps0w", bufs=3, space="PSUM") as ps0w,
            tc.tile_pool(name="nsp", bufs=2) as nsp,
        ):
            def p0(name):
                # one shared psum shape; 128-col (512B) slots per head, all
                # operands and outputs at base partition 0: runtime crashes
                # were traced to matmuls with base-64 operands.
                return ps0.tile([128, 512], F32, tag="ps0", name=name)

            # q_landT/k_landT: [d, h, L] = Wq_h-contract(x_landT) + bias
            for i, (dst, w, b) in enumerate(((qlT, wq, bqs), (klT, wk, bks))):
                p = p0(f"pl{i}")
                for h in range(NHG):
                    for ko in range(4):
                        nc.tensor.matmul(p[0:64, h * 128:h * 128 + L],
                                         lhsT=w[:, ko, bass.ts(h, 64)],
                                         rhs=xlT[:, ko, :],
                                         start=(ko == 0), stop=(ko == 3),
                                         skip_group_check=True)
                for h in range(NHG):
                    nc.scalar.activation(dst[:, h, :],
                                         p[0:64, h * 128:h * 128 + L],
                                         AF.Identity, bias=b[:, h, :])

            P0LVL = int(os.environ.get("K_P0LVL", "9"))
            # M1T = Wq-contract(k_landT)*SCALE*M8 (fp8), M3T symmetric.
            for im, (dstM, wT, landT) in enumerate(
                    ((M1T8, wqT, klT), (M3T8, wkT, qlT)) if P0LVL >= 2 else ()):
                for ko in range(4):
                    p = p0(f"pM{im}{ko}")
                    for h in range(NHG):
                        nc.tensor.matmul(
                            p[:, h * 128:h * 128 + 64],
                            lhsT=wT[:, h, bass.ts(ko, 128)],
                            rhs=landT[:, h, :],
                            start=True, stop=True, skip_group_check=True)
                    nc.vector.tensor_scalar_mul(
                        dstM[:, ko, :],
                        p[:].rearrange("p (h s) -> p h s", s=128)[:, :, 0:64],
                        SCALE * M8)

            if P0LVL < 3:
                W_holder["W"] = None
            # bias1[(h,l)] = sum_d bqs[(h,d)] k_landT[(h,d), l]
            # (matmul PSUM outputs must start at partition 0 on this compiler)
            if P0LVL >= 3:
              pb2 = p0("pb2")
            for h in range(NHG):
                t, psl = h // 2, bass.ts(h % 2, 64)
                nc.tensor.matmul(pb2[0:64, h:h + 1], lhsT=klT[psl, t, :],
                                 rhs=bqs[psl, t, :], start=True, stop=True,
                                 skip_group_check=True)
            for h in range(NHG):
                t, psl = h // 2, bass.ts(h % 2, 64)
                nc.vector.tensor_copy(bias1[psl, t, :], pb2[0:64, h:h + 1])

            # K2 per head: softmax(q_landT^T k_landT). Heads packed along
            # the free axis ([64, NHG, 64]) so every matmul writes psum
            # partition 0.
            K2 = nsp.tile([64, NHG, L], F32, tag="K2", name="K2")
            K2T = nsp.tile([64, NHG, L], F32, tag="K2T", name="K2T")
            k2e = small.tile([64, NHG, L], F32, tag="k2e")
            rs = small.tile([64, NHG, 1], F32, tag="k2rs")
            ri = small.tile([64, NHG, 1], F32, tag="k2ri")
            pk2 = p0("pk2")
            for h in range(NHG):
                t, psl = h // 2, bass.ts(h % 2, 64)
                nc.tensor.matmul(pk2[0:64, bass.ts(h, L)], lhsT=qlT[psl, t, :],
                                 rhs=klT[psl, t, :], start=True, stop=True,
                                 skip_group_check=True)
            for h in range(NHG):
                nc.scalar.activation(k2e[:, h, :], pk2[0:64, bass.ts(h, L)],
                                     AF.Exp, accum_out=rs[:, h, :])
            nc.vector.reciprocal(ri[:], rs[:])
            for h in range(NHG):
                nc.vector.tensor_scalar_mul(K2[:, h, :], k2e[:, h, :],
                                            ri[:, h, :])
            pt = p0("pk2t")
            for h in range(NHG):
                nc.tensor.transpose(pt[0:64, bass.ts(h, L)], K2[:, h, :],
                                    idf32[0:64, 0:64])
            nc.vector.tensor_copy(K2T[:], pt[0:64, 0:256])

            # N-S init: V = K2T/mx, W = K2/mx, mx = max col-sum of K2 per head
            mxi = nsp.tile([64, NHG, 1], F32, tag="mxi", name="mxi")
            cs = nsp.tile([64, NHG], F32, tag="cs", name="cs")
            nc.vector.reduce_sum(cs[:], K2T[:], axis=AX.X)
            for h in range(NHG):
                pc = p0(f"pc{h}")
                nc.tensor.transpose(pc[0:1, 0:64], cs[:, h:h + 1],
                                    idf32[0:64, 0:64])
                mx = nsp.tile([1, 1], F32, tag=f"mx{h}", name=f"mx{h}")
                nc.vector.reduce_max(mx[:], pc[0:1, 0:64], axis=AX.X)
                pb3 = p0(f"pb3{h}")
                nc.tensor.matmul(pb3[0:64, 0:1], lhsT=onesr[0:1, 0:64],
                                 rhs=mx[:], start=True, stop=True)
                nc.vector.reciprocal(mxi[:, h, :], pb3[0:64, 0:1])
            V = nsp.tile([64, NHG, L], F32, tag="V", name="V0")
            W = nsp.tile([64, NHG, L], F32, tag="W", name="W0")
            for h in range(NHG):
                nc.vector.tensor_scalar_mul(V[:, h, :], K2T[:, h, :],
                                            mxi[:, h, :])
                nc.vector.tensor_scalar_mul(W[:, h, :], K2[:, h, :],
                                            mxi[:, h, :])

            # N-S iterations: per-head matmuls into per-head psum columns,
            # one batched DVE op per stage.
            for it in range(N_ITER):
                last = it == N_ITER - 1
                T1 = nsp.tile([64, NHG, L], F32, tag="T1", name=f"T1_{it}")
                KVT = nsp.tile([64, NHG, L], F32, tag="KVT", name=f"KVT_{it}")
                T2 = nsp.tile([64, NHG, L], NSD[it], tag="T2", name=f"T2_{it}")
                T3 = nsp.tile([64, NHG, L], NSD[it], tag="T3", name=f"T3_{it}")
                Wn = nsp.tile([64, NHG, L], NSD[min(it + 1, N_ITER - 1)], tag="W", name=f"W_{it + 1}")
                pA = ps0w.tile([64, 512], F32, tag="ps0w", name=f"pA{it}")
                for h in range(NHG):
                    nc.tensor.matmul(pA[:, bass.ts(h, L)], lhsT=K2T[:, h, :],
                                     rhs=V[:, h, :], start=True, stop=True,
                                     skip_group_check=True)
                    nc.tensor.matmul(pA[:, 256 + h * L:256 + (h + 1) * L],
                                     lhsT=V[:, h, :], rhs=K2T[:, h, :],
                                     start=True, stop=True,
                                     skip_group_check=True)
                nc.vector.tensor_tensor(T1[:], nsc2[:, 0, :],
                                        pA[:, 0:256], op=OP.subtract)
                nc.vector.tensor_copy(KVT[:], pA[:, 256:512])
                pB = ps0w.tile([64, 512], F32, tag="ps0w", name=f"pB{it}")
                for h in range(NHG):
                    nc.tensor.matmul(pB[:, bass.ts(h, L)], lhsT=KVT[:, h, :],
                                     rhs=T1[:, h, :], start=True, stop=True,
                                     skip_group_check=True)
                nc.vector.tensor_tensor(T2[:], nsc2[:, 1, :],
                                        pB[:, 0:256], op=OP.subtract)
                pC = ps0w.tile([64, 512], F32, tag="ps0w", name=f"pC{it}")
                for h in range(NHG):
                    nc.tensor.matmul(pC[:, bass.ts(h, L)], lhsT=KVT[:, h, :],
                                     rhs=T2[:, h, :], start=True, stop=True,
                                     skip_group_check=True)
                nc.vector.scalar_tensor_tensor(
                    T3[:], pC[:, 0:256], -0.25,
                    nsc2[:, 2, :], op0=OP.mult, op1=OP.add)
                pD = ps0w.tile([64, 512], F32, tag="ps0w", name=f"pD{it}")
                for h in range(NHG):
                    nc.tensor.matmul(pD[:, 256 + h * L:256 + (h + 1) * L],
                                     lhsT=T3[:, h, :], rhs=W[:, h, :],
                                     start=True, stop=True,
                                     skip_group_check=True)
                    if not last:
                        nc.tensor.matmul(pD[:, bass.ts(h, L)], lhsT=W[:, h, :],
                                         rhs=T3[:, h, :], start=True, stop=True,
                                         skip_group_check=True)
                nc.vector.tensor_copy(Wn[:], pD[:, 256:512])
                if not last:
                    Vn = nsp.tile([64, NHG, L], F32, tag="V", name=f"V_{it + 1}")
                    nc.vector.tensor_copy(Vn[:], pD[:, 0:256])
                    V = Vn
                W = Wn
            W_f = small.tile([64, NHG, L], F32, tag="Wf")
            nc.vector.tensor_copy(W_f[:], W[:])
            W_holder["W"] = W_f

        if os.environ.get("K_NOA") == "1":
            with tc.tile_pool(name="zz2", bufs=1) as zz:
                z = zz.tile([128, E], BF16, tag="z")
                nc.vector.memset(z[:], 0.0)
                for c128 in range(64):
                    nc.sync.dma_start(out_d[bass.ts(c128, 128), :], z[:])
            return
        # ================= Phase A: v16 + kernel_3 =================
        # e3 kept resident; t1 runs as a standalone pass afterwards so the
        # phase-A psum budget leaves room for the interleaved N-S stages.
        e3sb = big.tile([128, 64, 256], BF16, tag="e3sb")
        with (
            tc.tile_pool(name="ps_v", bufs=3, space="PSUM") as ps_v,
            tc.tile_pool(name="ps_3", bufs=3, space="PSUM") as ps_3,
        ):
            for j in range(32):  # pairs of 128-seq chunks
                x8t = x8s[j // 8]
                xlot = xlos[j // 8]
                psv = ps_v.tile([128, 512], F32, tag="psv")
                ps3 = ps_3.tile([128, 512], F32, tag="ps3")
                for u in range(2):
                    c128 = 2 * j + u
                    sl = bass.ts(c128 % (SP // 128), 128)
                    usl = bass.ts(u, 256)
                    terms = ((x8t, wv16h), (x8t, wv16l), (xlot, wv16h))
                    i = 0
                    for xs, ws in terms:
                        for pr in range(2):
                            nc.tensor.matmul(
                                psv[:, usl],
                                lhsT=xs[:, 2 * pr:2 * pr + 2, sl],
                                rhs=ws[:, 2 * pr:2 * pr + 2, :],
                                start=(i == 0), stop=(i == 5),
                                perf_mode=PM.DoubleRow,
                                skip_group_check=True)
                            i += 1
                    for pr in range(2):
                        nc.tensor.matmul(
                            ps3[:, usl],
                            lhsT=x8t[:, 2 * pr:2 * pr + 2, sl],
                            rhs=M3T8[:, 2 * pr:2 * pr + 2, :],
                            start=(pr == 0), stop=(pr == 1),
                            perf_mode=PM.DoubleRow,
                            skip_group_check=True)
                if ns_stages:
                    ns_stages.pop(0)()
                vdst = vsb[:, 2 * j:2 * j + 2, :, 0:64]
                vsrc = psv[:].rearrange("p (c h d) -> p c h d", c=2, d=64)
                nc.vector.tensor_copy(vdst, vsrc)
                nc.scalar.activation(
                    e3sb[:, 2 * j:2 * j + 2, :].rearrange("p c m -> p (c m)"),
                    ps3[:], AF.Exp, scale=1.0 / M8)
            while ns_stages:
                ns_stages.pop(0)()
        ns_es.close()

        # ---- t1 accumulation pass ----
        with tc.tile_pool(name="ps_t1", bufs=4, space="PSUM") as ps_t1:
            t1ps = [ps_t1.tile([65, 64], F32, tag="t1", name=f"t1ps{h}")
                    for h in range(NHG)]
            for c128 in range(64):
                for h in range(NHG):
                    nc.tensor.matmul(
                        t1ps[h][:],
                        lhsT=vsb[:, c128, h, :],
                        rhs=e3sb[:, c128, h * 64:h * 64 + 64],
                        start=(c128 == 0), stop=(c128 == 63),
                        skip_group_check=True)
            # ======= Phase A': t1 -> t1n -> t2T -> m_sb =======
            with (
                tc.tile_pool(name="ps_m", bufs=2, space="PSUM") as ps_m,
                tc.tile_pool(name="ps_mE", bufs=2, space="PSUM") as ps_mE,
                tc.tile_pool(name="mp", bufs=2) as mp,
            ):
                W = W_holder["W"]
                t1n = mp.tile([64, NHG, 64], F32, tag="t1n", name="t1n")
                for h in range(NHG):
                    t1u = mp.tile([65, 64], F32, tag=f"t1u{h}", name=f"t1u{h}")
                    nc.vector.tensor_copy(t1u[:], t1ps[h][:])
                    ptt = ps_m.tile([64, 128], F32, tag="psm", name=f"ptt{h}")
                    nc.tensor.transpose(ptt[0:64, 0:65], t1u[:],
                                        idf32[0:65, 0:65])
                    d3i = mp.tile([64, 1], F32, tag=f"d3i{h}", name=f"d3i{h}")
                    nc.vector.reciprocal(d3i[:], ptt[0:64, 64:65])
                    nc.vector.tensor_scalar_mul(t1n[:, h, :],
                                                ptt[0:64, 0:64], d3i[:])
                nc.vector.tensor_tensor(t1n[:], t1n[:], bv16b[:],
                                        op=OP.add)
                t2T = mp.tile([64, NHG, 64], BF16, tag="t2T", name="t2T")
                pt2 = ps_mE.tile([64, 512], F32, tag="psmE", name="pt2")
                for h in range(NHG):
                    nc.tensor.matmul(pt2[:, h * 128:h * 128 + 64],
                                     lhsT=t1n[:, h, :], rhs=W[:, h, :],
                                     start=True, stop=True,
                                     skip_group_check=True)
                nc.vector.tensor_copy(
                    t2T[:], pt2[:].rearrange("p (h s) -> p h s",
                                             s=128)[:, :, 0:64])
                for h in range(NHG):
                    t, psl = h // 2, bass.ts(h % 2, 64)
                    pm_ = ps_mE.tile([64, 512], F32, tag="psmE", name=f"pm{h}")
                    nc.tensor.matmul(pm_[:], lhsT=t2T[:, h, :],
                                     rhs=wo16[:, h, :], start=True,
                                     stop=True)
                    nc.vector.tensor_copy(m_sb[psl, t, :], pm_[:])

        if os.environ.get("K_NOB") == "1":
            with tc.tile_pool(name="zz", bufs=1) as zz:
                z = zz.tile([128, E], BF16, tag="z")
                nc.vector.memset(z[:], 0.0)
                for c128 in range(64):
                    nc.sync.dma_start(out_d[bass.ts(c128, 128), :], z[:])
            return
        # ======= Phase B: kernel_1, normalize, output projection =======
        with (
            tc.tile_pool(name="ps_1", bufs=3, space="PSUM") as ps_1,
            tc.tile_pool(name="ps_r", bufs=2, space="PSUM") as ps_r,
            tc.tile_pool(name="ps_o", bufs=3, space="PSUM") as ps_o,
            tc.tile_pool(name="e1p", bufs=3) as e1p,
            tc.tile_pool(name="op", bufs=4) as op_,
        ):
            def emit_out(c, e1ns):
                for s4 in range(4):
                    pso = ps_o.tile([128, 512], F32, tag="pso")
                    for t in range(2):
                        nc.tensor.matmul(pso[:],
                                         lhsT=e1ns[t][:, bass.ts(s4, 128)],
                                         rhs=m_sb[:, t, :],
                                         start=(t == 0), stop=(t == 1))
                    osb = op_.tile([128, 512], BF16, tag="osb")
                    if s4 % 2 == 0:
                        nc.scalar.copy(osb[:], pso[:])
                    else:
                        nc.vector.tensor_copy(osb[:], pso[:])
                    nc.sync.dma_start(out_d[bass.ts(c * 4 + s4, 128), :],
                                      osb[:])

            prev = None
            for c in range(16):
                x8t = x8s[c // 4]
                sl = bass.ts(c % 4, 512)
                e1ns = []
                pss = []
                for t in range(2):
                    ps1 = ps_1.tile([128, 512], F32, tag="ps1")
                    if USE_DR:
                        for pr in range(2):
                            nc.tensor.matmul(
                                ps1[:],
                                lhsT=M1T8[:, 2 * pr:2 * pr + 2, bass.ts(t, 128)],
                                rhs=x8t[:, 2 * pr:2 * pr + 2, sl],
                                start=(pr == 0), stop=(pr == 1),
                                perf_mode=PM.DoubleRow)
                    else:
                        for ko in range(4):
                            nc.tensor.matmul(
                                ps1[:], lhsT=M1T8[:, ko, bass.ts(t, 128)],
                                rhs=x8t[:, ko, sl],
                                start=(ko == 0), stop=(ko == 3))
                    pss.append(ps1)
                if prev is not None:
                    emit_out(*prev)
                for t in range(2):
                    ps1 = pss[t]
                    e1 = e1p.tile([128, 512], BF16, tag="e1")
                    nc.scalar.activation(e1[:], ps1[:], AF.Exp,
                                         bias=bias1[:, t, :], scale=1.0 / M8)
                    psr = ps_r.tile([128, 512], F32, tag="psr")
                    nc.tensor.matmul(psr[:], lhsT=blk1[:], rhs=e1[:],
                                     start=True, stop=True)
                    rbs = e1p.tile([128, 512], BF16, tag="rbs")
                    with nc.allow_low_precision(reason="softmax rowsum recip"):
                        nc.vector.reciprocal(rbs[:], psr[:])
                    e1n = e1p.tile([128, 512], BF16, tag="e1n")
                    eng = nc.vector if (t == 0 or not USE_GPSIMD) else nc.gpsimd
                    eng.tensor_tensor(e1n[:], e1[:], rbs[:], op=OP.mult)
                    e1ns.append(e1n)
                prev = (c, e1ns)
            emit_out(*prev)


def _prep_inputs(x, Wq, bq, Wk, bk, Wv, bv, Wo, bo):
    bf = ml_dtypes.bfloat16
    f8 = ml_dtypes.float8_e4m3
    x = np.asarray(x, dtype=np.float32)
    Wq = np.asarray(Wq, dtype=np.float32)
    Wk = np.asarray(Wk, dtype=np.float32)
    Wv = np.asarray(Wv, dtype=np.float32)
    Wo = np.asarray(Wo, dtype=np.float32)
    bq = np.asarray(bq, dtype=np.float32)
    bk = np.asarray(bk, dtype=np.float32)
    bv = np.asarray(bv, dtype=np.float32)

    eye64 = np.eye(64, dtype=np.float32)
    nsc2 = np.stack([
        np.tile(c * eye64, (1, 4))
        for c in (7.0, 15.0, 3.25)
    ], axis=1)  # [64, 3, 256]
    consts = {
        "nsc2": np.ascontiguousarray(nsc2.astype(np.float32)),
        "idf32": np.eye(128, dtype=np.float32),
        "blk1": np.ascontiguousarray(
            np.kron(np.eye(2), np.ones((64, 64))).astype(bf)),
        "onesr": np.ones((1, 128), dtype=np.float32),
    }

    per_batch = []
    for b in range(4):
        xT = np.ascontiguousarray(x[b].T)                      # [E, S] f32
        x8 = xT.astype(f8)
        xlo = (xT - x8.astype(np.float32)).astype(f8)
        xlT = np.ascontiguousarray(
            (x[b].reshape(64, 128, E).mean(axis=1).T * SCALE).astype(bf))
        per_batch.append((x8, xlo, xlT))

    in_maps = []
    for core in range(8):
        b, g = core // 2, core % 2
        hsl = slice(g * 256, (g + 1) * 256)
        x8, xlo, xlT = per_batch[b]
        wv16 = Wv[:, hsl] * 16.0
        wv16h = wv16.astype(f8)
        wv16l = (wv16 - wv16h.astype(np.float32)).astype(f8)
        # bv*16 broadcast over the 64 L partitions, [64, NHG, 64]
        bv16 = (bv[hsl] * 16.0).reshape(4, 64)
        bv16b = np.broadcast_to(bv16[None, :, :], (64, 4, 64)).astype(np.float32)
        bv16b = np.ascontiguousarray(bv16b)
        in_maps.append({
            "x8": x8, "xlo": xlo, "xlT": xlT,
            "wq": np.ascontiguousarray(Wq[:, hsl]).astype(bf),
            "wk": np.ascontiguousarray(Wk[:, hsl]).astype(bf),
            "wqT": np.ascontiguousarray(Wq[:, hsl].T).astype(bf),  # [256,E] -> (h p) m
            "wkT": np.ascontiguousarray(Wk[:, hsl].T).astype(bf),
            "wv16h": np.ascontiguousarray(wv16h),
            "wv16l": np.ascontiguousarray(wv16l),
            "wo16": np.ascontiguousarray(Wo[hsl, :] / 16.0).astype(bf),
            "bqs": np.ascontiguousarray(bq[hsl] * SCALE).astype(bf),
            "bks": np.ascontiguousarray(bk[hsl] * SCALE).astype(bf),
            "bv16b": bv16b,
            **consts,
        })
    return in_maps


def run_on_device(in_maps, **kwargs):
    global _CACHED_NC
    if _CACHED_NC is None:
        _CACHED_NC = _build()
    return run_bass_kernel_spmd(_CACHED_NC, in_maps, core_ids=list(range(8)),
                                **kwargs)


def kernel(x, Wq, bq, Wk, bk, Wv, bv, Wo, bo):
    in_maps = _prep_inputs(x, Wq, bq, Wk, bk, Wv, bv, Wo, bo)
    res = run_on_device(in_maps)
    bo = np.asarray(bo, dtype=np.float32)
    out = np.empty((4, S, E), dtype=np.float32)
    for b in range(4):
        out[b] = (res.results[2 * b]["out"].astype(np.float32)
                  + res.results[2 * b + 1]["out"].astype(np.float32) + bo)
    return out


# revision 44
# speedup vs baseline: 1.9192x; 1.0087x over previous
"""Nystrom attention Trainium2 kernel (fused landmark formulation).

Sharding: 8 cores = 4 batches x 2 head-groups (4 heads each). Each core
computes its (batch, head-group) slice; the host sums the two bf16 partial
output projections per batch (in f32) and adds bo.

Algebra (per head h, SCALE = HEAD_DIM**-0.25, q = x@Wq + bq etc.):
  x_land   = segment means of x (host; linear pooling of the input)
  q_landT  = Wq^T-contract(x_landT*SCALE) + bq*SCALE   [(h,d), L] on device
  logits1  = x @ M1T + bq.k_land         M1T = Wq-contract(k_landT)*SCALE
  logits3  = (x @ M3T)^T-ish             (kernel_3's bias is constant along
                                          its softmax axis; drops out exactly)
  K2       = softmax(q_landT^T k_landT) per head; invK2 via Newton-Schulz.
  v16      = x @ (Wv*16)   (bv folds into t1n; Wo/16 compensates the 16)
  t1       = [v16|1]^T @ exp(logits3) -> t1n = rows/rowsum + bv*16
  m        = ((invK2 @ t1n) @ (Wo/16))^T-chain => m_sb [(h,L), E]
  out      = (e1 / rowsum_head(e1)) @ m,   e1 = exp(logits1)

Big matmuls are fp8e4 DoubleRow (hi/lo compensated for v; logits are tiny
(~0.1 rms) so single fp8 is safe there). Newton-Schulz runs bf16 for
iterations 0-4 and f32 for the final iteration (last-iter precision
dominates the result), with its stages interleaved into phase A.
"""

import os
import numpy as np
import ml_dtypes

import concourse.bass as bass
import concourse.tile as tile
from concourse import bacc, mybir
from concourse.bass_utils import run_bass_kernel_spmd

BF16 = mybir.dt.bfloat16
F32 = mybir.dt.float32
FP8 = mybir.dt.float8e4
AF = mybir.ActivationFunctionType
AX = mybir.AxisListType
OP = mybir.AluOpType
PM = mybir.MatmulPerfMode

S = 8192        # sequence length
E = 512         # embedding dim
D = 64          # head dim
L = 64          # landmarks
NHG = 4         # heads per core (head group)
N_ITER = 6
SCALE = 1.0 / np.sqrt(np.sqrt(D))
M8 = 64.0       # fp8 prescale on M1T/M3T (undone by exp scale)
NSPLIT = 4      # x8/xlo DMA pipelining splits along S
USE_DR = os.environ.get("K_DR", "1") == "1"
USE_GPSIMD = os.environ.get("K_GP", "1") == "1"

_CACHED_NC = None
_TILES = {}


def _build():
    nc = bacc.Bacc("TRN2", target_bir_lowering=False, debug=False, num_devices=8)

    dram = {}
    for name, shape, dt in [
        ("x8", [E, S], FP8),
        ("xlo", [E, S], FP8),
        ("blob0", [E, 576], BF16),
        ("blob1", [256, 2 * E + 2], BF16),
        ("wv16h", [E, 256], FP8),
        ("wv16l", [E, 256], FP8),
        ("wo16", [256, E], BF16),

        ("bv16b", [64, NHG, L], F32),
        ("nsc2", [64, 3, 256], F32),
        ("idf32", [128, 128], F32),
        ("blk1", [128, 128], BF16),
        ("onesr", [1, 128], F32),
    ]:
        dram[name] = nc.dram_tensor(name, shape, dt, kind="ExternalInput").ap()
    out_d = nc.dram_tensor("out", [S, E], BF16, kind="ExternalOutput").ap()

    with tile.TileContext(nc) as tc:
        _emit(nc, tc, dram, out_d)
    nc.compile()
    return nc


def _emit(nc, tc, dram, out_d):
    SP = S // NSPLIT
    with (
        tc.tile_pool(name="const", bufs=1) as const,
        tc.tile_pool(name="big", bufs=1) as big,
        tc.tile_pool(name="small", bufs=2) as small,
    ):
        def load(name, shape, dt, pat=None, **kw):
            t = const.tile(shape, dt, tag=name)
            src = dram[name]
            if pat is not None:
                src = src.rearrange(pat, **kw)
            nc.sync.dma_start(t[:], src)
            return t

        # small consts first so phase 0 can begin immediately
        blob0 = load("blob0", [128, 4, 576], BF16, "(ko p) m -> p ko m",
                     p=128)
        xlT = blob0[:, :, 0:64]
        wq = blob0[:, :, 64:320]
        wk = blob0[:, :, 320:576]
        blob1 = load("blob1", [64, NHG, 2 * E + 2], BF16,
                     "(h p) m -> p h m", p=64)
        wqT = blob1[:, :, 0:E]
        wkT = blob1[:, :, E:2 * E]
        bqs = blob1[:, :, 2 * E:2 * E + 1]
        bks = blob1[:, :, 2 * E + 1:2 * E + 2]
        idf32 = load("idf32", [128, 128], F32)
        onesr = load("onesr", [1, 128], F32)
        wv16h = load("wv16h", [128, 4, 256], FP8, "(ko p) m -> p ko m", p=128)
        wv16l = load("wv16l", [128, 4, 256], FP8, "(ko p) m -> p ko m", p=128)

        # big inputs, split along S; first split lands before the consts that
        # are only needed later (N-S constants, phase A'/B weights) so the
        # streaming phase can start as early as possible.
        def load_split(i):
            ssl = slice(i * SP, (i + 1) * SP)
            t8 = big.tile([128, 4, SP], FP8, tag=f"x8_{i}")
            nc.sync.dma_start(
                t8[:], dram["x8"][:, ssl].rearrange("(ko p) s -> p ko s", p=128))
            tlo = big.tile([128, 4, SP], FP8, tag=f"xlo_{i}")
            nc.sync.dma_start(
                tlo[:], dram["xlo"][:, ssl].rearrange("(ko p) s -> p ko s", p=128))
            return t8, tlo

        x8s, xlos = [], []
        t8, tlo = load_split(0)
        x8s.append(t8)
        xlos.append(tlo)
        nsc2 = load("nsc2", [64, 3, 256], F32)
        bv16b = load("bv16b", [64, NHG, L], F32)
        blk1 = load("blk1", [128, 128], BF16)
        wo16 = load("wo16", [64, NHG, E], BF16, "(h p) m -> p h m", p=64)
        for i in range(1, NSPLIT):
            t8, tlo = load_split(i)
            x8s.append(t8)
            xlos.append(tlo)

        # persistent activations / results
        vsb = big.tile([128, 64, NHG, 65], BF16, tag="v")  # (s | c4, head, d+1)
        nc.vector.memset(vsb[:, :, :, 64:65], 1.0)
        m_sb = const.tile([128, 2, E], BF16, tag="m")      # out-proj matrix
        bias1 = const.tile([128, 2, 1], F32, tag="bias1")  # exp bias per (h,L)

        qlT = const.tile([64, NHG, L], BF16, tag="qlT")  # (d part, head, L)
        klT = const.tile([64, NHG, L], BF16, tag="klT")
        M1T8 = const.tile([128, 4, 256], FP8, tag="M1T8")  # (E part, hL free)
        M3T8 = const.tile([128, 4, 256], FP8, tag="M3T8")

        if os.environ.get("K_NOP0") == "1":
            with tc.tile_pool(name="zz3", bufs=1) as zz:
                z = zz.tile([128, E], BF16, tag="z")
                nc.vector.memset(z[:], 0.0)
                for c128 in range(64):
                    nc.sync.dma_start(out_d[bass.ts(c128, 128), :], z[:])
            return
        # ============ Phase 0: landmark branch + N-S inverse ============
        global _TILES
        _TILES = {"vsb": vsb, "m_sb": m_sb, "qlT": qlT, "klT": klT,
                  "M1T8": M1T8, "M3T8": M3T8, "bias1": bias1}
        W_holder = {}
        with (
            tc.tile_pool(name="ps0", bufs=4, space="PSUM") as ps0,
            tc.tile_pool(name="ps0w", bufs=3, space="PSUM") as ps0w,
            tc.tile_pool(name="nsp", bufs=2) as nsp,
        ):
            def p0(name):
                # one shared psum shape; 128-col (512B) slots per head, all
                # operands and outputs at base partition 0: runtime crashes
                # were traced to matmuls with base-64 operands.
                return ps0.tile([128, 512], F32, tag="ps0", name=name)

            # q_landT/k_landT: [d, h, L] = Wq_h-contract(x_landT) + bias
            for i, (dst, w, b) in enumerate(((qlT, wq, bqs), (klT, wk, bks))):
                p = p0(f"pl{i}")
                for h in range(NHG):
                    for ko in range(4):
                        nc.tensor.matmul(p[0:64, h * 128:h * 128 + L],
                                         lhsT=w[:, ko, bass.ts(h, 64)],
                                         rhs=xlT[:, ko, :],
                                         start=(ko == 0), stop=(ko == 3),
                                         skip_group_check=True)
                for h in range(NHG):
                    nc.scalar.activation(dst[:, h, :],
                                         p[0:64, h * 128:h * 128 + L],
                                         AF.Identity, bias=b[:, h, :])

            P0LVL = int(os.environ.get("K_P0LVL", "9"))
            # M1T = Wq-contract(k_landT)*SCALE*M8 (fp8), M3T symmetric.
            for im, (dstM, wT, landT) in enumerate(
                    ((M1T8, wqT, klT), (M3T8, wkT, qlT)) if P0LVL >= 2 else ()):
                for ko in range(4):
                    p = p0(f"pM{im}{ko}")
                    for h in range(NHG):
                        nc.tensor.matmul(
                            p[:, h * 128:h * 128 + 64],
                            lhsT=wT[:, h, bass.ts(ko, 128)],
                            rhs=landT[:, h, :],
                            start=True, stop=True, skip_group_check=True)
                    nc.vector.tensor_scalar_mul(
                        dstM[:, ko, :],
                        p[:].rearrange("p (h s) -> p h s", s=128)[:, :, 0:64],
                        SCALE * M8)

            if P0LVL < 3:
                W_holder["W"] = None
            # bias1[(h,l)] = sum_d bqs[(h,d)] k_landT[(h,d), l]
            # (matmul PSUM outputs must start at partition 0 on this compiler)
            if P0LVL >= 3:
              pb2 = p0("pb2")
            for h in range(NHG):
                t, psl = h // 2, bass.ts(h % 2, 64)
                nc.tensor.matmul(pb2[0:64, h:h + 1], lhsT=klT[psl, t, :],
                                 rhs=bqs[psl, t, :], start=True, stop=True,
                                 skip_group_check=True)
            for h in range(NHG):
                t, psl = h // 2, bass.ts(h % 2, 64)
                nc.vector.tensor_copy(bias1[psl, t, :], pb2[0:64, h:h + 1])

            # K2 per head: softmax(q_landT^T k_landT). Heads packed along
            # the free axis ([64, NHG, 64]) so every matmul writes psum
            # partition 0.
            K2 = nsp.tile([64, NHG, L], F32, tag="K2", name="K2")
            K2T = nsp.tile([64, NHG, L], F32, tag="K2T", name="K2T")
            k2e = small.tile([64, NHG, L], F32, tag="k2e")
            rs = small.tile([64, NHG, 1], F32, tag="k2rs")
            ri = small.tile([64, NHG, 1], F32, tag="k2ri")
            pk2 = p0("pk2")
            for h in range(NHG):
                t, psl = h // 2, bass.ts(h % 2, 64)
                nc.tensor.matmul(pk2[0:64, bass.ts(h, L)], lhsT=qlT[psl, t, :],
                                 rhs=klT[psl, t, :], start=True, stop=True,
                                 skip_group_check=True)
            for h in range(NHG):
                nc.scalar.activation(k2e[:, h, :], pk2[0:64, bass.ts(h, L)],
                                     AF.Exp, accum_out=rs[:, h, :])
            nc.vector.reciprocal(ri[:], rs[:])
            for h in range(NHG):
                nc.vector.tensor_scalar_mul(K2[:, h, :], k2e[:, h, :],
                                            ri[:, h, :])
            pt = p0("pk2t")
            for h in range(NHG):
                nc.tensor.transpose(pt[0:64, bass.ts(h, L)], K2[:, h, :],
                                    idf32[0:64, 0:64])
            nc.vector.tensor_copy(K2T[:], pt[0:64, 0:256])

            # N-S init: V = K2T/mx, W = K2/mx, mx = max col-sum of K2 per head
            mxi = nsp.tile([64, NHG, 1], F32, tag="mxi", name="mxi")
            cs = nsp.tile([64, NHG], F32, tag="cs", name="cs")
            nc.vector.reduce_sum(cs[:], K2T[:], axis=AX.X)
            for h in range(NHG):
                pc = p0(f"pc{h}")
                nc.tensor.transpose(pc[0:1, 0:64], cs[:, h:h + 1],
                                    idf32[0:64, 0:64])
                mx = nsp.tile([1, 1], F32, tag=f"mx{h}", name=f"mx{h}")
                nc.vector.reduce_max(mx[:], pc[0:1, 0:64], axis=AX.X)
                pb3 = p0(f"pb3{h}")
                nc.tensor.matmul(pb3[0:64, 0:1], lhsT=onesr[0:1, 0:64],
                                 rhs=mx[:], start=True, stop=True)
                nc.vector.reciprocal(mxi[:, h, :], pb3[0:64, 0:1])
            V = nsp.tile([64, NHG, L], F32, tag="V", name="V0")
            W = nsp.tile([64, NHG, L], F32, tag="W", name="W0")
            for h in range(NHG):
                nc.vector.tensor_scalar_mul(V[:, h, :], K2T[:, h, :],
                                            mxi[:, h, :])
                nc.vector.tensor_scalar_mul(W[:, h, :], K2[:, h, :],
                                            mxi[:, h, :])

            # N-S iterations: per-head matmuls into per-head psum columns,
            # one batched DVE op per stage.
            for it in range(N_ITER):
                last = it == N_ITER - 1
                T1 = nsp.tile([64, NHG, L], F32, tag="T1", name=f"T1_{it}")
                KVT = nsp.tile([64, NHG, L], F32, tag="KVT", name=f"KVT_{it}")
                T2 = nsp.tile([64, NHG, L], NSD[it], tag="T2", name=f"T2_{it}")
                T3 = nsp.tile([64, NHG, L], NSD[it], tag="T3", name=f"T3_{it}")
                Wn = nsp.tile([64, NHG, L], NSD[min(it + 1, N_ITER - 1)], tag="W", name=f"W_{it + 1}")
                pA = ps0w.tile([64, 512], F32, tag="ps0w", name=f"pA{it}")
                for h in range(NHG):
                    nc.tensor.matmul(pA[:, bass.ts(h, L)], lhsT=K2T[:, h, :],
                                     rhs=V[:, h, :], start=True, stop=True,
                                     skip_group_check=True)
                    nc.tensor.matmul(pA[:, 256 + h * L:256 + (h + 1) * L],
                                     lhsT=V[:, h, :], rhs=K2T[:, h, :],
                                     start=True, stop=True,
                                     skip_group_check=True)
                nc.vector.tensor_tensor(T1[:], nsc2[:, 0, :],
                                        pA[:, 0:256], op=OP.subtract)
                nc.vector.tensor_copy(KVT[:], pA[:, 256:512])
                pB = ps0w.tile([64, 512], F32, tag="ps0w", name=f"pB{it}")
                for h in range(NHG):
                    nc.tensor.matmul(pB[:, bass.ts(h, L)], lhsT=KVT[:, h, :],
                                     rhs=T1[:, h, :], start=True, stop=True,
                                     skip_group_check=True)
                nc.vector.tensor_tensor(T2[:], nsc2[:, 1, :],
                                        pB[:, 0:256], op=OP.subtract)
                pC = ps0w.tile([64, 512], F32, tag="ps0w", name=f"pC{it}")
                for h in range(NHG):
                    nc.tensor.matmul(pC[:, bass.ts(h, L)], lhsT=KVT[:, h, :],
                                     rhs=T2[:, h, :], start=True, stop=True,
                                     skip_group_check=True)
                nc.vector.scalar_tensor_tensor(
                    T3[:], pC[:, 0:256], -0.25,
                    nsc2[:, 2, :], op0=OP.mult, op1=OP.add)
                pD = ps0w.tile([64, 512], F32, tag="ps0w", name=f"pD{it}")
                for h in range(NHG):
                    nc.tensor.matmul(pD[:, 256 + h * L:256 + (h + 1) * L],
                                     lhsT=T3[:, h, :], rhs=W[:, h, :],
                                     start=True, stop=True,
                                     skip_group_check=True)
                    if not last:
                        nc.tensor.matmul(pD[:, bass.ts(h, L)], lhsT=W[:, h, :],
                                         rhs=T3[:, h, :], start=True, stop=True,
                                         skip_group_check=True)
                nc.vector.tensor_copy(Wn[:], pD[:, 256:512])
                if not last:
                    Vn = nsp.tile([64, NHG, L], F32, tag="V", name=f"V_{it + 1}")
                    nc.vector.tensor_copy(Vn[:], pD[:, 0:256])
                    V = Vn
                W = Wn
            W_f = small.tile([64, NHG, L], F32, tag="Wf")
            nc.vector.tensor_copy(W_f[:], W[:])
            W_holder["W"] = W_f

        if os.environ.get("K_NOA") == "1":
            with tc.tile_pool(name="zz2", bufs=1) as zz:
                z = zz.tile([128, E], BF16, tag="z")
                nc.vector.memset(z[:], 0.0)
                for c128 in range(64):
                    nc.sync.dma_start(out_d[bass.ts(c128, 128), :], z[:])
            return
        # ================= Phase A: v16 + kernel_3 =================
        # e3 kept resident; t1 runs as a standalone pass afterwards so the
        # phase-A psum budget leaves room for the interleaved N-S stages.
        e3sb = big.tile([128, 64, 256], BF16, tag="e3sb")
        with (
            tc.tile_pool(name="ps_v", bufs=3, space="PSUM") as ps_v,
            tc.tile_pool(name="ps_3", bufs=3, space="PSUM") as ps_3,
        ):
            for j in range(32):  # pairs of 128-seq chunks
                x8t = x8s[j // 8]
                xlot = xlos[j // 8]
                psv = ps_v.tile([128, 512], F32, tag="psv")
                ps3 = ps_3.tile([128, 512], F32, tag="ps3")
                for u in range(2):
                    c128 = 2 * j + u
                    sl = bass.ts(c128 % (SP // 128), 128)
                    usl = bass.ts(u, 256)
                    terms = ((x8t, wv16h), (x8t, wv16l), (xlot, wv16h))
                    i = 0
                    for xs, ws in terms:
                        for pr in range(2):
                            nc.tensor.matmul(
                                psv[:, usl],
                                lhsT=xs[:, 2 * pr:2 * pr + 2, sl],
                                rhs=ws[:, 2 * pr:2 * pr + 2, :],
                                start=(i == 0), stop=(i == 5),
                                perf_mode=PM.DoubleRow,
                                skip_group_check=True)
                            i += 1
                    for pr in range(2):
                        nc.tensor.matmul(
                            ps3[:, usl],
                            lhsT=x8t[:, 2 * pr:2 * pr + 2, sl],
                            rhs=M3T8[:, 2 * pr:2 * pr + 2, :],
                            start=(pr == 0), stop=(pr == 1),
                            perf_mode=PM.DoubleRow,
                            skip_group_check=True)
                if ns_stages:
                    ns_stages.pop(0)()
                vdst = vsb[:, 2 * j:2 * j + 2, :, 0:64]
                vsrc = psv[:].rearrange("p (c h d) -> p c h d", c=2, d=64)
                nc.vector.tensor_copy(vdst, vsrc)
                nc.scalar.activation(
                    e3sb[:, 2 * j:2 * j + 2, :].rearrange("p c m -> p (c m)"),
                    ps3[:], AF.Exp, scale=1.0 / M8)
            while ns_stages:
                ns_stages.pop(0)()
        ns_es.close()

        # ---- t1 accumulation pass ----
        with tc.tile_pool(name="ps_t1", bufs=4, space="PSUM") as ps_t1:
            t1ps = [ps_t1.tile([65, 64], F32, tag="t1", name=f"t1ps{h}")
                    for h in range(NHG)]
            for c128 in range(64):
                for h in range(NHG):
                    nc.tensor.matmul(
                        t1ps[h][:],
                        lhsT=vsb[:, c128, h, :],
                        rhs=e3sb[:, c128, h * 64:h * 64 + 64],
                        start=(c128 == 0), stop=(c128 == 63),
                        skip_group_check=True)
            # ======= Phase A': t1 -> t1n -> t2T -> m_sb =======
            with (
                tc.tile_pool(name="ps_m", bufs=2, space="PSUM") as ps_m,
                tc.tile_pool(name="ps_mE", bufs=2, space="PSUM") as ps_mE,
                tc.tile_pool(name="mp", bufs=2) as mp,
            ):
                W = W_holder["W"]
                t1n = mp.tile([64, NHG, 64], F32, tag="t1n", name="t1n")
                for h in range(NHG):
                    t1u = mp.tile([65, 64], F32, tag=f"t1u{h}", name=f"t1u{h}")
                    nc.vector.tensor_copy(t1u[:], t1ps[h][:])
                    ptt = ps_m.tile([64, 128], F32, tag="psm", name=f"ptt{h}")
                    nc.tensor.transpose(ptt[0:64, 0:65], t1u[:],
                                        idf32[0:65, 0:65])
                    d3i = mp.tile([64, 1], F32, tag=f"d3i{h}", name=f"d3i{h}")
                    nc.vector.reciprocal(d3i[:], ptt[0:64, 64:65])
                    nc.vector.tensor_scalar_mul(t1n[:, h, :],
                                                ptt[0:64, 0:64], d3i[:])
                nc.vector.tensor_tensor(t1n[:], t1n[:], bv16b[:],
                                        op=OP.add)
                t2T = mp.tile([64, NHG, 64], BF16, tag="t2T", name="t2T")
                pt2 = ps_mE.tile([64, 512], F32, tag="psmE", name="pt2")
                for h in range(NHG):
                    nc.tensor.matmul(pt2[:, h * 128:h * 128 + 64],
                                     lhsT=t1n[:, h, :], rhs=W[:, h, :],
                                     start=True, stop=True,
                                     skip_group_check=True)
                nc.vector.tensor_copy(
                    t2T[:], pt2[:].rearrange("p (h s) -> p h s",
                                             s=128)[:, :, 0:64])
                for h in range(NHG):
                    t, psl = h // 2, bass.ts(h % 2, 64)
                    pm_ = ps_mE.tile([64, 512], F32, tag="psmE", name=f"pm{h}")
                    nc.tensor.matmul(pm_[:], lhsT=t2T[:, h, :],
                                     rhs=wo16[:, h, :], start=True,
                                     stop=True)
                    nc.vector.tensor_copy(m_sb[psl, t, :], pm_[:])

        if os.environ.get("K_NOB") == "1":
            with tc.tile_pool(name="zz", bufs=1) as zz:
                z = zz.tile([128, E], BF16, tag="z")
                nc.vector.memset(z[:], 0.0)
                for c128 in range(64):
                    nc.sync.dma_start(out_d[bass.ts(c128, 128), :], z[:])
            return
        # ======= Phase B: kernel_1, normalize, output projection =======
        with (
            tc.tile_pool(name="ps_1", bufs=3, space="PSUM") as ps_1,
            tc.tile_pool(name="ps_r", bufs=2, space="PSUM") as ps_r,
            tc.tile_pool(name="ps_o", bufs=3, space="PSUM") as ps_o,
            tc.tile_pool(name="e1p", bufs=3) as e1p,
            tc.tile_pool(name="op", bufs=4) as op_,
        ):
            def emit_out(c, e1ns):
                for s4 in range(4):
                    pso = ps_o.tile([128, 512], F32, tag="pso")
                    for t in range(2):
                        nc.tensor.matmul(pso[:],
                                         lhsT=e1ns[t][:, bass.ts(s4, 128)],
                                         rhs=m_sb[:, t, :],
                                         start=(t == 0), stop=(t == 1))
                    osb = op_.tile([128, 512], BF16, tag="osb")
                    on_act = s4 % 2 == 0
                    if on_act:
                        nc.scalar.copy(osb[:], pso[:])
                    else:
                        nc.vector.tensor_copy(osb[:], pso[:])
                    nc.sync.dma_start(out_d[bass.ts(c * 4 + s4, 128), :],
                                      osb[:])

            prev = None
            for c in range(16):
                x8t = x8s[c // 4]
                sl = bass.ts(c % 4, 512)
                e1ns = []
                pss = []
                for t in range(2):
                    ps1 = ps_1.tile([128, 512], F32, tag="ps1")
                    if USE_DR:
                        for pr in range(2):
                            nc.tensor.matmul(
                                ps1[:],
                                lhsT=M1T8[:, 2 * pr:2 * pr + 2, bass.ts(t, 128)],
                                rhs=x8t[:, 2 * pr:2 * pr + 2, sl],
                                start=(pr == 0), stop=(pr == 1),
                                perf_mode=PM.DoubleRow)
                    else:
                        for ko in range(4):
                            nc.tensor.matmul(
                                ps1[:], lhsT=M1T8[:, ko, bass.ts(t, 128)],
                                rhs=x8t[:, ko, sl],
                                start=(ko == 0), stop=(ko == 3))
                    pss.append(ps1)
                if prev is not None:
                    emit_out(*prev)
                for t in range(2):
                    ps1 = pss[t]
                    e1 = e1p.tile([128, 512], BF16, tag="e1")
                    nc.scalar.activation(e1[:], ps1[:], AF.Exp,
                                         bias=bias1[:, t, :], scale=1.0 / M8)
                    psr = ps_r.tile([128, 512], F32, tag="psr")
                    nc.tensor.matmul(psr[:], lhsT=blk1[:], rhs=e1[:],
                                     start=True, stop=True)
                    rbs = e1p.tile([128, 512], BF16, tag="rbs")
                    with nc.allow_low_precision(reason="softmax rowsum recip"):
                        nc.vector.reciprocal(rbs[:], psr[:])
                    e1n = e1p.tile([128, 512], BF16, tag="e1n")
                    eng = nc.vector if (t == 0 or not USE_GPSIMD) else nc.gpsimd
                    eng.tensor_tensor(e1n[:], e1[:], rbs[:], op=OP.mult)
                    e1ns.append(e1n)
                prev = (c, e1ns)
            emit_out(*prev)


def _prep_inputs(x, Wq, bq, Wk, bk, Wv, bv, Wo, bo):
    bf = ml_dtypes.bfloat16
    f8 = ml_dtypes.float8_e4m3
    x = np.asarray(x, dtype=np.float32)
    Wq = np.asarray(Wq, dtype=np.float32)
    Wk = np.asarray(Wk, dtype=np.float32)
    Wv = np.asarray(Wv, dtype=np.float32)
    Wo = np.asarray(Wo, dtype=np.float32)
    bq = np.asarray(bq, dtype=np.float32)
    bk = np.asarray(bk, dtype=np.float32)
    bv = np.asarray(bv, dtype=np.float32)

    eye64 = np.eye(64, dtype=np.float32)
    nsc2 = np.stack([
        np.tile(c * eye64, (1, 4))
        for c in (7.0, 15.0, 3.25)
    ], axis=1)  # [64, 3, 256]
    consts = {
        "nsc2": np.ascontiguousarray(nsc2.astype(np.float32)),
        "idf32": np.eye(128, dtype=np.float32),
        "blk1": np.ascontiguousarray(
            np.kron(np.eye(2), np.ones((64, 64))).astype(bf)),
        "onesr": np.ones((1, 128), dtype=np.float32),
    }

    per_batch = []
    for b in range(4):
        xT = np.ascontiguousarray(x[b].T)                      # [E, S] f32
        x8 = xT.astype(f8)
        xlo = (xT - x8.astype(np.float32)).astype(f8)
        xlT = np.ascontiguousarray(
            (x[b].reshape(64, 128, E).mean(axis=1).T * SCALE).astype(bf))
        per_batch.append((x8, xlo, xlT))

    in_maps = []
    for core in range(8):
        b, g = core // 2, core % 2
        hsl = slice(g * 256, (g + 1) * 256)
        x8, xlo, xlT = per_batch[b]
        wv16 = Wv[:, hsl] * 16.0
        wv16h = wv16.astype(f8)
        wv16l = (wv16 - wv16h.astype(np.float32)).astype(f8)
        # bv*16 broadcast over the 64 L partitions, [64, NHG, 64]
        bv16 = (bv[hsl] * 16.0).reshape(4, 64)
        bv16b = np.broadcast_to(bv16[None, :, :], (64, 4, 64)).astype(np.float32)
        bv16b = np.ascontiguousarray(bv16b)
        blob0 = np.concatenate(
            [xlT, Wq[:, hsl].astype(np.float32), Wk[:, hsl]], axis=1)
        in_maps.append({
            "x8": x8, "xlo": xlo,
            "blob0": np.ascontiguousarray(blob0).astype(bf),
            "blob1": np.ascontiguousarray(np.concatenate(
                [Wq[:, hsl].T, Wk[:, hsl].T,
                 (bq[hsl] * SCALE)[:, None], (bk[hsl] * SCALE)[:, None]],
                axis=1)).astype(bf),
            "wv16h": np.ascontiguousarray(wv16h),
            "wv16l": np.ascontiguousarray(wv16l),
            "wo16": np.ascontiguousarray(Wo[hsl, :] / 16.0).astype(bf),

            "bv16b": bv16b,
            **consts,
        })
    return in_maps


def run_on_device(in_maps, **kwargs):
    global _CACHED_NC
    if _CACHED_NC is None:
        _CACHED_NC = _build()
    return run_bass_kernel_spmd(_CACHED_NC, in_maps, core_ids=list(range(8)),
                                **kwargs)


def kernel(x, Wq, bq, Wk, bk, Wv, bv, Wo, bo):
    in_maps = _prep_inputs(x, Wq, bq, Wk, bk, Wv, bv, Wo, bo)
    res = run_on_device(in_maps)
    bo = np.asarray(bo, dtype=np.float32)
    out = np.empty((4, S, E), dtype=np.float32)
    for b in range(4):
        out[b] = (res.results[2 * b]["out"].astype(np.float32)
                  + res.results[2 * b + 1]["out"].astype(np.float32) + bo)
    return out
